# revision 13
# baseline (speedup 1.0000x reference)
"""GroupInfoNCE loss kernel for 8 Trainium2 NeuronCores.

Strategy (row-sharded, fused, collective-free):
  - Core k owns rows [1024k, 1024k+1024) of S = scale * f1n @ f2n.T.
  - f2 is passed to core k pre-rotated by -1024k rows so each core's
    diagonal (positive) block sits at local column offset 0; same NEFF
    on all cores, different data.
  - The 8192x8192 logits matrix never touches HBM: each [128,1024] GEMM
    tile is consumed in PSUM by ScalarE exp (the per-row
    scale*rsqrt(|f1_i|^2) is folded into the activation scale AP).
  - Engine balance: squares on GpSimd/Pool, sumsq reduce + row block
    sums on DVE, exp/ln on ScalarE pinned to the natural_log_exp
    activation table (one explicit LoadActFuncSet, zero reloads), GEMM
    + column block sums on PE, bf16 casts on Pool.
  - f2 is processed in 4 quarters, with the per-quarter norm scales and
    casts interleaved into the main loop's engine program order so prep
    pipelines with the GEMM instead of serializing ahead of it.
  - One merged input (f1 shard ++ rotated f2), one merged output, and
    NEFF-inline constants minimize per-dispatch argument overhead.
  - Host does the tiny O(GN) combine.
"""

import os
import numpy as np

GN, D = 8192, 256
NGRP = 16               # group length N
EPS = 0.1               # label smoothing
G = GN // NGRP          # 512 groups
NCORES = 8
RPC = GN // NCORES      # 1024 rows per core
NSTRIP = RPC // 128     # 8 strips of 128 rows
NJB = GN // 1024        # 8 j-blocks of 1024 columns
NCH = GN // 128         # 64 f2 chunks of 128 rows

ACT_TABLE_LN_EXP = 6    # act_info.json index of natural_log_exp_and_others

_cache = {}
last_results = None


def _build_program(ln_s: float, parts: int = 5, repeat: int = 1):
    # parts: 1=prep only, 2=+gemm+exp, 3=+rowred, 4=+colsum, >=5 full
    from contextlib import ExitStack
    import concourse.bass as bass  # noqa: F401
    import concourse.mybir as mybir
    import concourse.tile as tile
    from concourse import bacc

    f32 = mybir.dt.float32
    bf16 = mybir.dt.bfloat16
    AF = mybir.ActivationFunctionType
    ALU = mybir.AluOpType
    AX = mybir.AxisListType

    nc = bacc.Bacc(
        "TRN2",
        target_bir_lowering=False,
        debug=False,
        enable_asserts=False,
        num_devices=NCORES,
    )

    # rows 0..1023: f1 shard; rows 1024..9215: rotated f2
    feats_d = nc.dram_tensor("feats", [RPC + GN, D], f32, kind="ExternalInput").ap()
    # rows 0..63: column block sums; row 64 cols 0:3072: [128,3,8] small stats
    o_all_d = nc.dram_tensor("o_all", [65, GN], f32, kind="ExternalOutput").ap()

    ones64_np, mask128_np = _constants()
    ones64_d = nc.inline_tensor(np.ascontiguousarray(ones64_np), name="ones64").ap()
    mask128_d = nc.inline_tensor(np.ascontiguousarray(mask128_np), name="mask128").ap()

    with tile.TileContext(nc) as tc, ExitStack() as ctx:
        singles = ctx.enter_context(tc.tile_pool(name="singles", bufs=1))
        sqp = ctx.enter_context(tc.tile_pool(name="sqp", bufs=3))
        expp = ctx.enter_context(tc.tile_pool(name="expp", bufs=4))
        psg = ctx.enter_context(tc.tile_pool(name="psg", bufs=2, space="PSUM"))
        psc = ctx.enter_context(tc.tile_pool(name="psc", bufs=2, space="PSUM"))

        ones64_sb = singles.tile([128, NSTRIP, 64], bf16, name="ones64_sb")
        mask128_sb = singles.tile([128, 8], f32, name="mask128_sb")

        f1all = singles.tile([128, NSTRIP, D], f32, name="f1all")
        f2all = [
            singles.tile([128, 8, D], f32, name=f"f2all{jb}", tag=f"f2all{jb}")
            for jb in range(NJB)
        ]
        f1bh = [
            singles.tile([128, NSTRIP, 128], bf16, name=f"f1bh{h}", tag=f"f1bh{h}")
            for h in (0, 1)
        ]
        f2bh = [
            [
                singles.tile([128, 8, 128], bf16, name=f"f2bh{jb}_{h}", tag=f"f2bh{jb}_{h}")
                for h in (0, 1)
            ]
            for jb in range(NJB)
        ]
        f1T = [
            singles.tile([128, RPC], bf16, name=f"f1T{h}", tag=f"f1T{h}")
            for h in (0, 1)
        ]
        f2T = [
            [
                singles.tile([128, 8, 128], bf16, name=f"f2T{jb}_{h}", tag=f"f2T{jb}_{h}")
                for h in (0, 1)
            ]
            for jb in range(NJB)
        ]
        ssq_all = singles.tile([128, NSTRIP + NCH], f32, name="ssq_all")
        lssq_all = singles.tile([128, NSTRIP + NCH], f32, name="lssq_all")
        scales = singles.tile([128, NSTRIP + NCH], f32, name="scales")
        lns_sb = singles.tile([128, 1], f32, name="lns_sb")
        rowblk = [
            singles.tile([128, G], f32, name=f"rowblk{t}", tag=f"rowblk{t}")
            for t in range(NSTRIP)
        ]
        o_small_sb = singles.tile([128, 3, NSTRIP], f32, name="o_small_sb")

        nc.sync.dma_start(out=ones64_sb, in_=ones64_d)
        nc.sync.dma_start(out=mask128_sb, in_=mask128_d)
        nc.vector.memset(lns_sb, ln_s)

        # pin the one activation table that covers exp/ln/square/copy
        nc.scalar.add_instruction(mybir.InstLoadActFuncSet(
            name=nc.get_next_instruction_name(),
            act_func_set_id=ACT_TABLE_LN_EXP, ins=[], outs=[]))

        def load_jb(jb):
            nc.sync.dma_start(
                out=f2all[jb],
                in_=feats_d[RPC + jb * 1024 : RPC + (jb + 1) * 1024, :].rearrange(
                    "(t p) d -> p t d", p=128
                ),
            )

        def sq_jb(jb):
            for cc in range(8):
                c = jb * 8 + cc
                sq = sqp.tile([128, D], f32, tag="sq", name="sq")
                nc.gpsimd.tensor_mul(sq, f2all[jb][:, cc, :], f2all[jb][:, cc, :])
                nc.vector.reduce_sum(
                    out=ssq_all[:, NSTRIP + c : NSTRIP + c + 1], in_=sq, axis=AX.X
                )

        def scales_jb(jb):
            lo = NSTRIP + 8 * jb
            nc.scalar.activation(
                lssq_all[:, lo : lo + 8], ssq_all[:, lo : lo + 8], AF.Ln
            )
            nc.scalar.activation(
                scales[:, lo : lo + 8], lssq_all[:, lo : lo + 8], AF.Exp,
                scale=-0.5,
            )

        def cast_jb(jb):
            for cc in range(8):
                c = jb * 8 + cc
                for h in (0, 1):
                    nc.gpsimd.tensor_scalar_mul(
                        f2bh[jb][h][:, cc, :],
                        f2all[jb][:, cc, h * 128 : (h + 1) * 128],
                        scales[:, NSTRIP + c : NSTRIP + c + 1],
                    )
            for h in (0, 1):
                nc.sync.dma_start_transpose(f2T[jb][h], f2bh[jb][h])

        def prep_jb(jb):
            sq_jb(jb)
            scales_jb(jb)
            cast_jb(jb)

        def rowblk_pool(t, jb, expb):
            # 16-wide block sums via pairwise-add tree on the Pool engine
            e = expb.rearrange("p (g n) -> p g n", n=NGRP)
            s8 = sqp.tile([128, G // 8, 8], f32, tag="s8", name="s8")
            nc.gpsimd.tensor_add(s8, e[:, :, 0:8], e[:, :, 8:16])
            s4 = sqp.tile([128, G // 8, 4], f32, tag="s4", name="s4")
            nc.gpsimd.tensor_add(s4, s8[:, :, 0:4], s8[:, :, 4:8])
            s2 = sqp.tile([128, G // 8, 2], f32, tag="s2", name="s2")
            nc.gpsimd.tensor_add(s2, s4[:, :, 0:2], s4[:, :, 2:4])
            nc.gpsimd.tensor_add(
                rowblk[t][:, jb * 64 : (jb + 1) * 64],
                s2[:, :, 0:1].rearrange("p a b -> p (a b)"),
                s2[:, :, 1:2].rearrange("p a b -> p (a b)"),
            )

        for _rep in range(repeat):
            # ---------------- head: jb0 + f1 race to feed the first GEMM ------
            load_jb(0)
            nc.sync.dma_start(
                out=f1all, in_=feats_d[:RPC, :].rearrange("(t p) d -> p t d", p=128)
            )
            load_jb(1)
            sq_jb(0)
            scales_jb(0)
            for t in range(NSTRIP):
                sq1 = sqp.tile([128, D], bf16, tag="sq1", name="sq1")
                nc.scalar.activation(
                    sq1, f1all[:, t, :], AF.Square,
                    accum_out=ssq_all[:, t : t + 1],
                )
                for h in (0, 1):
                    nc.gpsimd.tensor_copy(
                        f1bh[h][:, t, :], f1all[:, t, h * 128 : (h + 1) * 128]
                    )
            nc.scalar.activation(
                lssq_all[:, :NSTRIP], ssq_all[:, :NSTRIP], AF.Ln
            )
            nc.scalar.activation(
                scales[:, :NSTRIP], lssq_all[:, :NSTRIP], AF.Exp,
                scale=-0.5, bias=lns_sb,
            )
            for h in (0, 1):
                nc.sync.dma_start_transpose(
                    f1T[h].rearrange("p (t j) -> p t j", j=128), f1bh[h]
                )

            # ---------------- f2 head: jb0, jb1 ready before main loop --------
            cast_jb(0)
            load_jb(2)
            prep_jb(1)

            # ---------------- main loop, prep for jb+2 interleaved ------------
            for jb in range(NJB if parts >= 2 else 0):
                if jb + 3 < NJB:
                    load_jb(jb + 3)
                if jb + 2 < NJB:
                    prep_jb(jb + 2)

                colps = psc.tile([64, 1024], f32, tag="colps", name="colps") if parts >= 4 else None
                rhs = [f2T[jb][h].rearrange("p t j -> p (t j)") for h in (0, 1)]
                for t in range(NSTRIP):
                    ps = psg.tile([128, 1024], f32, tag="gemm", name="ps")
                    for h2 in (0, 1):
                        for kc in (0, 1):
                            nc.tensor.matmul(
                                ps[:, h2 * 512 : (h2 + 1) * 512],
                                lhsT=f1T[kc][:, t * 128 : (t + 1) * 128],
                                rhs=rhs[kc][:, h2 * 512 : (h2 + 1) * 512],
                                start=(kc == 0),
                                stop=(kc == 1),
                            )
                    expb = expp.tile([128, 1024], bf16, tag="exp", name="expb")
                    nc.scalar.activation(
                        expb, ps, AF.Exp, scale=scales[:, t : t + 1]
                    )
                    if parts >= 3:
                        if t < 2:
                            rowblk_pool(t, jb, expb)
                        else:
                            nc.vector.reduce_sum(
                                out=rowblk[t][:, jb * 64 : (jb + 1) * 64],
                                in_=expb.rearrange("p (g n) -> p g n", n=NGRP),
                                axis=AX.X,
                            )
                    if parts >= 4:
                        for h2 in (0, 1):
                            nc.tensor.matmul(
                                colps[:, h2 * 512 : (h2 + 1) * 512],
                                lhsT=ones64_sb[:, t, :],
                                rhs=expb[:, h2 * 512 : (h2 + 1) * 512],
                                start=(t == 0),
                                stop=(t == NSTRIP - 1),
                            )
                if parts >= 4:
                    crawj = expp.tile([64, 1024], f32, tag="crawj", name="crawj")
                    if jb % 2 == 0:
                        nc.scalar.copy(crawj, colps)
                    else:
                        nc.vector.tensor_copy(crawj, colps)
                    nc.sync.dma_start(
                        out=o_all_d[:64, jb * 1024 : (jb + 1) * 1024], in_=crawj
                    )

            # ---------------- per-strip tails ---------------------------------
            for t in range(NSTRIP if parts >= 5 else 0):
                nc.vector.reduce_sum(
                    out=o_small_sb[:, 0, t : t + 1], in_=rowblk[t], axis=AX.X
                )
                nc.scalar.activation(
                    rowblk[t], rowblk[t], AF.Ln,
                    accum_out=o_small_sb[:, 1, t : t + 1],
                )
                posscr = sqp.tile([128, 8], f32, tag="posscr", name="posscr")
                nc.gpsimd.tensor_mul(
                    posscr, rowblk[t][:, t * 8 : (t + 1) * 8], mask128_sb
                )
                nc.vector.reduce_sum(
                    out=o_small_sb[:, 2, t : t + 1], in_=posscr, axis=AX.X
                )
            if parts >= 5:
                nc.sync.dma_start(
                    out=o_all_d[64:65, : 3 * NSTRIP * 128].rearrange(
                        "a (p x) -> (a p) x", p=128
                    ),
                    in_=o_small_sb,
                )

    nc.compile()
    return nc


def _constants():
    import ml_dtypes

    p = np.arange(128)
    ones64 = np.zeros((128, NSTRIP, 64), dtype=ml_dtypes.bfloat16)
    for t in range(NSTRIP):
        ones64[p, t, 8 * t + p // 16] = 1.0
    mask128 = np.zeros((128, 8), dtype=np.float32)
    mask128[p, p // 16] = 1.0
    return ones64, mask128


def make_in_maps(f1, f2):
    return [
        {
            "feats": np.ascontiguousarray(
                np.concatenate(
                    [f1[k * RPC : (k + 1) * RPC], np.roll(f2, -k * RPC, axis=0)]
                )
            ),
        }
        for k in range(NCORES)
    ]


def kernel(image_features1, image_features2, logit_scale):
    global last_results
    from concourse.bass_utils import run_bass_kernel_spmd

    f1 = np.ascontiguousarray(np.asarray(image_features1, dtype=np.float32))
    f2 = np.ascontiguousarray(np.asarray(image_features2, dtype=np.float32))
    s = float(np.asarray(logit_scale).reshape(-1)[0])

    key = round(np.log(s), 12)
    if key not in _cache:
        _cache[key] = _build_program(float(np.log(s)))
    nc = _cache[key]

    in_maps = make_in_maps(f1, f2)

    try:
        res = run_bass_kernel_spmd(
            nc,
            in_maps,
            core_ids=list(range(NCORES)),
            trace=bool(os.environ.get("KTRACE")),
        )
    except ModuleNotFoundError:
        # axon build without NTFF profiling hooks — rerun without trace
        res = run_bass_kernel_spmd(
            nc, in_maps, core_ids=list(range(NCORES)), trace=False
        )
    last_results = res

    # ---------------- host combine (O(GN) work) ----------------
    eps = EPS
    S1 = 0.0
    for k in range(NCORES):
        o_all = res.results[k]["o_all"].astype(np.float64)
        small = o_all[64, : 3 * NSTRIP * 128].reshape(128, 3, NSTRIP)
        asum = small[:, 0, :]  # sum_j exp
        slog = small[:, 1, :]  # sum_g log blocksum
        pos = small[:, 2, :]   # log blocksum at positive block
        per_row = np.log(asum) - (1.0 - eps) * pos - (eps / G) * slog
        S1 += per_row.sum()

    j = np.arange(GN)
    a_tot = np.zeros(GN, dtype=np.float64)
    b_tot = np.zeros(GN, dtype=np.float64)
    pos2 = np.zeros(GN, dtype=np.float64)
    for k in range(NCORES):
        craw = res.results[k]["o_all"][:64].astype(np.float64)  # [64, GN]
        jj = (j - k * RPC) % GN
        cg = craw[:, jj]  # columns reindexed to global j
        a_tot += cg.sum(axis=0)
        b_tot += np.log(cg).sum(axis=0)
        jr = np.arange(k * RPC, (k + 1) * RPC)
        pos2[jr] = craw[(jr // 16) % 64, jr % RPC]
    per_row2 = np.log(a_tot) - (1.0 - eps) * np.log(pos2) - (eps / G) * b_tot
    S2 = per_row2.sum()

    loss = (S1 + S2) / (2.0 * GN)
    return np.array(loss, dtype=np.float32)


# revision 17
# speedup vs baseline: 1.2519x; 1.2519x over previous
"""GroupInfoNCE loss kernel for 8 Trainium2 NeuronCores.

Strategy (row-sharded, fused, collective-free):
  - Core k owns rows [1024k, 1024k+1024) of S = scale * f1n @ f2n.T.
  - f2 is passed to core k pre-rotated by -1024k rows so each core's
    diagonal (positive) block sits at local column offset 0; same NEFF
    on all cores, different data.
  - The 8192x8192 logits matrix never touches HBM: each [128,1024] GEMM
    tile is consumed in PSUM by ScalarE exp (the per-row
    scale*rsqrt(|f1_i|^2) is folded into the activation scale AP).
  - Engine balance: squares on GpSimd/Pool, sumsq reduce + row block
    sums on DVE, exp/ln on ScalarE pinned to the natural_log_exp
    activation table (one explicit LoadActFuncSet, zero reloads), GEMM
    + column block sums on PE, bf16 casts on Pool.
  - f2 is processed in 4 quarters, with the per-quarter norm scales and
    casts interleaved into the main loop's engine program order so prep
    pipelines with the GEMM instead of serializing ahead of it.
  - One merged input (f1 shard ++ rotated f2), one merged output, and
    NEFF-inline constants minimize per-dispatch argument overhead.
  - Host does the tiny O(GN) combine.
"""

import os
import numpy as np

GN, D = 8192, 256
NGRP = 16               # group length N
EPS = 0.1               # label smoothing
G = GN // NGRP          # 512 groups
NCORES = 8
RPC = GN // NCORES      # 1024 rows per core
NSTRIP = RPC // 128     # 8 strips of 128 rows
NJB = GN // 1024        # 8 j-blocks of 1024 columns
NCH = GN // 128         # 64 f2 chunks of 128 rows

ACT_TABLE_LN_EXP = 6    # act_info.json index of natural_log_exp_and_others

_cache = {}
last_results = None


def _build_program(ln_s: float, parts: int = 5, repeat: int = 1,
                   psg_bufs: int = 3, psc_bufs: int = 1, expp_bufs: int = 4,
                   kc_outer: bool = False, defer_colsum: bool = False):
    # parts: 1=prep only, 2=+gemm+exp, 3=+rowred, 4=+colsum, >=5 full
    from contextlib import ExitStack
    import concourse.bass as bass  # noqa: F401
    import concourse.mybir as mybir
    import concourse.tile as tile
    from concourse import bacc

    f32 = mybir.dt.float32
    bf16 = mybir.dt.bfloat16
    AF = mybir.ActivationFunctionType
    ALU = mybir.AluOpType
    AX = mybir.AxisListType

    nc = bacc.Bacc(
        "TRN2",
        target_bir_lowering=False,
        debug=False,
        enable_asserts=False,
        num_devices=NCORES,
    )

    # rows 0..1023: f1 shard; rows 1024..9215: rotated f2
    feats_d = nc.dram_tensor("feats", [RPC + GN, D], f32, kind="ExternalInput").ap()
    # rows 0..63: column block sums; row 64 cols 0:3072: [128,3,8] small stats
    o_all_d = nc.dram_tensor("o_all", [65, GN], f32, kind="ExternalOutput").ap()

    ones64_np, mask128_np = _constants()
    ones64_d = nc.inline_tensor(np.ascontiguousarray(ones64_np), name="ones64").ap()
    mask128_d = nc.inline_tensor(np.ascontiguousarray(mask128_np), name="mask128").ap()

    with tile.TileContext(nc) as tc, ExitStack() as ctx:
        singles = ctx.enter_context(tc.tile_pool(name="singles", bufs=1))
        sqp = ctx.enter_context(tc.tile_pool(name="sqp", bufs=3))
        expp = ctx.enter_context(tc.tile_pool(name="expp", bufs=expp_bufs))
        psg = ctx.enter_context(tc.tile_pool(name="psg", bufs=psg_bufs, space="PSUM"))
        psc = ctx.enter_context(tc.tile_pool(name="psc", bufs=psc_bufs, space="PSUM"))

        ones64_sb = singles.tile([128, NSTRIP, 64], bf16, name="ones64_sb")
        mask128_sb = singles.tile([128, 8], f32, name="mask128_sb")

        f1all = singles.tile([128, NSTRIP, D], f32, name="f1all")
        f2all = [
            singles.tile([128, 8, D], f32, name=f"f2all{jb}", tag=f"f2all{jb}")
            for jb in range(NJB)
        ]
        f1bh = [
            singles.tile([128, NSTRIP, 128], bf16, name=f"f1bh{h}", tag=f"f1bh{h}")
            for h in (0, 1)
        ]
        f2bh = [
            [
                singles.tile([128, 8, 128], bf16, name=f"f2bh{jb}_{h}", tag=f"f2bh{jb}_{h}")
                for h in (0, 1)
            ]
            for jb in range(NJB)
        ]
        f1T = [
            singles.tile([128, RPC], bf16, name=f"f1T{h}", tag=f"f1T{h}")
            for h in (0, 1)
        ]
        f2T = [
            [
                singles.tile([128, 8, 128], bf16, name=f"f2T{jb}_{h}", tag=f"f2T{jb}_{h}")
                for h in (0, 1)
            ]
            for jb in range(NJB)
        ]
        ssq_all = singles.tile([128, NSTRIP + NCH], f32, name="ssq_all")
        lssq_all = singles.tile([128, NSTRIP + NCH], f32, name="lssq_all")
        scales = singles.tile([128, NSTRIP + NCH], f32, name="scales")
        lns_sb = singles.tile([128, 1], f32, name="lns_sb")
        rowblk = [
            singles.tile([128, G], f32, name=f"rowblk{t}", tag=f"rowblk{t}")
            for t in range(NSTRIP)
        ]
        o_small_sb = singles.tile([128, 3, NSTRIP], f32, name="o_small_sb")

        nc.sync.dma_start(out=ones64_sb, in_=ones64_d)
        nc.sync.dma_start(out=mask128_sb, in_=mask128_d)
        nc.vector.memset(lns_sb, ln_s)

        # pin the one activation table that covers exp/ln/square/copy
        nc.scalar.add_instruction(mybir.InstLoadActFuncSet(
            name=nc.get_next_instruction_name(),
            act_func_set_id=ACT_TABLE_LN_EXP, ins=[], outs=[]))

        def load_jb(jb):
            nc.sync.dma_start(
                out=f2all[jb],
                in_=feats_d[RPC + jb * 1024 : RPC + (jb + 1) * 1024, :].rearrange(
                    "(t p) d -> p t d", p=128
                ),
            )

        def sq_jb(jb):
            for cc in range(8):
                c = jb * 8 + cc
                sq = sqp.tile([128, D], f32, tag="sq", name="sq")
                nc.gpsimd.tensor_mul(sq, f2all[jb][:, cc, :], f2all[jb][:, cc, :])
                nc.vector.reduce_sum(
                    out=ssq_all[:, NSTRIP + c : NSTRIP + c + 1], in_=sq, axis=AX.X
                )

        def scales_jb(jb):
            lo = NSTRIP + 8 * jb
            nc.scalar.activation(
                lssq_all[:, lo : lo + 8], ssq_all[:, lo : lo + 8], AF.Ln
            )
            nc.scalar.activation(
                scales[:, lo : lo + 8], lssq_all[:, lo : lo + 8], AF.Exp,
                scale=-0.5,
            )

        def cast_jb(jb):
            for cc in range(8):
                c = jb * 8 + cc
                for h in (0, 1):
                    nc.gpsimd.tensor_scalar_mul(
                        f2bh[jb][h][:, cc, :],
                        f2all[jb][:, cc, h * 128 : (h + 1) * 128],
                        scales[:, NSTRIP + c : NSTRIP + c + 1],
                    )
            for h in (0, 1):
                nc.sync.dma_start_transpose(f2T[jb][h], f2bh[jb][h])

        def prep_jb(jb):
            sq_jb(jb)
            scales_jb(jb)
            cast_jb(jb)

        def rowblk_pool(t, jb, expb):
            # 16-wide block sums via pairwise-add tree on the Pool engine
            e = expb.rearrange("p (g n) -> p g n", n=NGRP)
            s8 = sqp.tile([128, G // 8, 8], f32, tag="s8", name="s8")
            nc.gpsimd.tensor_add(s8, e[:, :, 0:8], e[:, :, 8:16])
            s4 = sqp.tile([128, G // 8, 4], f32, tag="s4", name="s4")
            nc.gpsimd.tensor_add(s4, s8[:, :, 0:4], s8[:, :, 4:8])
            s2 = sqp.tile([128, G // 8, 2], f32, tag="s2", name="s2")
            nc.gpsimd.tensor_add(s2, s4[:, :, 0:2], s4[:, :, 2:4])
            nc.gpsimd.tensor_add(
                rowblk[t][:, jb * 64 : (jb + 1) * 64],
                s2[:, :, 0:1].rearrange("p a b -> p (a b)"),
                s2[:, :, 1:2].rearrange("p a b -> p (a b)"),
            )

        for _rep in range(repeat):
            # ---------------- head: jb0 + f1 race to feed the first GEMM ------
            load_jb(0)
            nc.sync.dma_start(
                out=f1all, in_=feats_d[:RPC, :].rearrange("(t p) d -> p t d", p=128)
            )
            load_jb(1)
            sq_jb(0)
            scales_jb(0)
            for t in range(NSTRIP):
                sq1 = sqp.tile([128, D], bf16, tag="sq1", name="sq1")
                nc.scalar.activation(
                    sq1, f1all[:, t, :], AF.Square,
                    accum_out=ssq_all[:, t : t + 1],
                )
                for h in (0, 1):
                    nc.gpsimd.tensor_copy(
                        f1bh[h][:, t, :], f1all[:, t, h * 128 : (h + 1) * 128]
                    )
            nc.scalar.activation(
                lssq_all[:, :NSTRIP], ssq_all[:, :NSTRIP], AF.Ln
            )
            nc.scalar.activation(
                scales[:, :NSTRIP], lssq_all[:, :NSTRIP], AF.Exp,
                scale=-0.5, bias=lns_sb,
            )
            for h in (0, 1):
                nc.sync.dma_start_transpose(
                    f1T[h].rearrange("p (t j) -> p t j", j=128), f1bh[h]
                )

            # ---------------- f2 head: jb0, jb1 ready before main loop --------
            cast_jb(0)
            load_jb(2)
            prep_jb(1)

            # ---------------- main loop, prep for jb+2 interleaved ------------
            # parts 6: gemm only; 7: +exp const-scale; 8: +exp scale-AP f32 out
            main_jbs = NJB if (2 <= parts < 10 or parts in (6, 7, 8)) else 0
            for jb in range(main_jbs):
                if jb + 3 < NJB:
                    load_jb(jb + 3)
                if jb + 2 < NJB:
                    prep_jb(jb + 2)

                colps = psc.tile([64, 1024], f32, tag="colps", name="colps") if 4 <= parts < 10 else None
                rhs = [f2T[jb][h].rearrange("p t j -> p (t j)") for h in (0, 1)]
                expbs = []
                for t in range(NSTRIP):
                    ps = psg.tile([128, 1024], f32, tag="gemm", name="ps")
                    loop = (
                        [(h2, kc) for kc in (0, 1) for h2 in (0, 1)]
                        if kc_outer else
                        [(h2, kc) for h2 in (0, 1) for kc in (0, 1)]
                    )
                    for h2, kc in loop:
                        nc.tensor.matmul(
                            ps[:, h2 * 512 : (h2 + 1) * 512],
                            lhsT=f1T[kc][:, t * 128 : (t + 1) * 128],
                            rhs=rhs[kc][:, h2 * 512 : (h2 + 1) * 512],
                            start=(kc == 0),
                            stop=(kc == 1),
                        )
                    if parts == 6:
                        continue
                    if parts == 7:
                        expb = expp.tile([128, 1024], bf16, tag="exp", name="expb")
                        nc.scalar.activation(expb, ps, AF.Exp, scale=0.07)
                        continue
                    if parts == 8:
                        expb32 = expp.tile([128, 1024], f32, tag="exp32", name="expb32")
                        nc.scalar.activation(
                            expb32, ps, AF.Exp, scale=scales[:, t : t + 1]
                        )
                        continue
                    expb = expp.tile([128, 1024], bf16, tag="exp", name="expb")
                    nc.scalar.activation(
                        expb, ps, AF.Exp, scale=scales[:, t : t + 1]
                    )
                    if 3 <= parts < 10:
                        if t < 2:
                            rowblk_pool(t, jb, expb)
                        else:
                            nc.vector.reduce_sum(
                                out=rowblk[t][:, jb * 64 : (jb + 1) * 64],
                                in_=expb.rearrange("p (g n) -> p g n", n=NGRP),
                                axis=AX.X,
                            )
                    if 4 <= parts < 10 and defer_colsum:
                        expbs.append(expb)
                    elif 4 <= parts < 10:
                        for h2 in (0, 1):
                            nc.tensor.matmul(
                                colps[:, h2 * 512 : (h2 + 1) * 512],
                                lhsT=ones64_sb[:, t, :],
                                rhs=expb[:, h2 * 512 : (h2 + 1) * 512],
                                start=(t == 0),
                                stop=(t == NSTRIP - 1),
                            )
                if 4 <= parts < 10 and defer_colsum:
                    for t in range(NSTRIP):
                        for h2 in (0, 1):
                            nc.tensor.matmul(
                                colps[:, h2 * 512 : (h2 + 1) * 512],
                                lhsT=ones64_sb[:, t, :],
                                rhs=expbs[t][:, h2 * 512 : (h2 + 1) * 512],
                                start=(t == 0),
                                stop=(t == NSTRIP - 1),
                            )
                if 4 <= parts < 10:
                    crawj = expp.tile([64, 1024], f32, tag="crawj", name="crawj")
                    if jb % 2 == 0:
                        nc.scalar.copy(crawj, colps)
                    else:
                        nc.vector.tensor_copy(crawj, colps)
                    nc.sync.dma_start(
                        out=o_all_d[:64, jb * 1024 : (jb + 1) * 1024], in_=crawj
                    )

            # ---------------- per-strip tails ---------------------------------
            for t in range(NSTRIP if 5 <= parts < 10 else 0):
                nc.vector.reduce_sum(
                    out=o_small_sb[:, 0, t : t + 1], in_=rowblk[t], axis=AX.X
                )
                nc.scalar.activation(
                    rowblk[t], rowblk[t], AF.Ln,
                    accum_out=o_small_sb[:, 1, t : t + 1],
                )
                posscr = sqp.tile([128, 8], f32, tag="posscr", name="posscr")
                nc.gpsimd.tensor_mul(
                    posscr, rowblk[t][:, t * 8 : (t + 1) * 8], mask128_sb
                )
                nc.vector.reduce_sum(
                    out=o_small_sb[:, 2, t : t + 1], in_=posscr, axis=AX.X
                )
            if 5 <= parts < 10:
                nc.sync.dma_start(
                    out=o_all_d[64:65, : 3 * NSTRIP * 128].rearrange(
                        "a (p x) -> (a p) x", p=128
                    ),
                    in_=o_small_sb,
                )

    nc.compile()
    return nc


def _constants():
    import ml_dtypes

    p = np.arange(128)
    ones64 = np.zeros((128, NSTRIP, 64), dtype=ml_dtypes.bfloat16)
    for t in range(NSTRIP):
        ones64[p, t, 8 * t + p // 16] = 1.0
    mask128 = np.zeros((128, 8), dtype=np.float32)
    mask128[p, p // 16] = 1.0
    return ones64, mask128


def make_in_maps(f1, f2):
    return [
        {
            "feats": np.ascontiguousarray(
                np.concatenate(
                    [f1[k * RPC : (k + 1) * RPC], np.roll(f2, -k * RPC, axis=0)]
                )
            ),
        }
        for k in range(NCORES)
    ]


def kernel(image_features1, image_features2, logit_scale):
    global last_results
    from concourse.bass_utils import run_bass_kernel_spmd

    f1 = np.ascontiguousarray(np.asarray(image_features1, dtype=np.float32))
    f2 = np.ascontiguousarray(np.asarray(image_features2, dtype=np.float32))
    s = float(np.asarray(logit_scale).reshape(-1)[0])

    key = round(np.log(s), 12)
    if key not in _cache:
        _cache[key] = _build_program(float(np.log(s)))
    nc = _cache[key]

    in_maps = make_in_maps(f1, f2)

    try:
        res = run_bass_kernel_spmd(
            nc,
            in_maps,
            core_ids=list(range(NCORES)),
            trace=bool(os.environ.get("KTRACE")),
        )
    except ModuleNotFoundError:
        # axon build without NTFF profiling hooks — rerun without trace
        res = run_bass_kernel_spmd(
            nc, in_maps, core_ids=list(range(NCORES)), trace=False
        )
    last_results = res

    # ---------------- host combine (O(GN) work) ----------------
    eps = EPS
    S1 = 0.0
    for k in range(NCORES):
        o_all = res.results[k]["o_all"].astype(np.float64)
        small = o_all[64, : 3 * NSTRIP * 128].reshape(128, 3, NSTRIP)
        asum = small[:, 0, :]  # sum_j exp
        slog = small[:, 1, :]  # sum_g log blocksum
        pos = small[:, 2, :]   # log blocksum at positive block
        per_row = np.log(asum) - (1.0 - eps) * pos - (eps / G) * slog
        S1 += per_row.sum()

    j = np.arange(GN)
    a_tot = np.zeros(GN, dtype=np.float64)
    b_tot = np.zeros(GN, dtype=np.float64)
    pos2 = np.zeros(GN, dtype=np.float64)
    for k in range(NCORES):
        craw = res.results[k]["o_all"][:64].astype(np.float64)  # [64, GN]
        jj = (j - k * RPC) % GN
        cg = craw[:, jj]  # columns reindexed to global j
        a_tot += cg.sum(axis=0)
        b_tot += np.log(cg).sum(axis=0)
        jr = np.arange(k * RPC, (k + 1) * RPC)
        pos2[jr] = craw[(jr // 16) % 64, jr % RPC]
    per_row2 = np.log(a_tot) - (1.0 - eps) * np.log(pos2) - (eps / G) * b_tot
    S2 = per_row2.sum()

    loss = (S1 + S2) / (2.0 * GN)
    return np.array(loss, dtype=np.float32)
